# revision 42
# baseline (speedup 1.0000x reference)
# Trainium2 Bass kernel: 2:4 structured activation pruning + Linear.
#
#   out = magnitude_prune_2of4(x.reshape(-1, 4096)) @ weight.T
#
# Sharding: data-parallel over the flattened token dim (16384 tokens ->
# 2048/core across 8 cores); weight replicated. No collectives.
#
# Layout trick: the host casts x to bf16 and permutes the d axis so the
# four group positions land in four contiguous 1024-wide blocks, ordered
# [x0, x2, x1, x3].  Every DVE op in the 2:4 threshold tree then runs on
# packed stride-1 bf16 operands (2x_1p DVE mode), with no strided or
# broadcast access patterns.  The same permutation is applied to the
# weight rows on the host, so the GEMM is unchanged.
#
# Per-core pipeline, per 128-token tile:
#   DMA x (bf16) -> ACT Abs -> DVE stock max/min tree (packed, 2x) ->
#   per-group 2nd-max threshold (exact bf16) -> custom DVE prune
#   (|x| >= thr ? x : 0) -> DMA XBAR transpose SBUF->SBUF (bf16) ->
#   PE bf16 matmul accumulating over 32 d-chunks -> ACT PSUM->SBUF ->
#   DMA out.  The PE does nothing but the GEMM (no on-PE transposes).
import numpy as np

N_CORES = 8
BS, SEQ, D = 4, 4096, 4096
OUTF = 1024
TOK_TOTAL = BS * SEQ
TOK = TOK_TOTAL // N_CORES      # 2048 tokens per core
P = 128                         # SBUF partitions
NT = TOK // P                   # 16 token tiles per core
NCH = D // P                    # 32 d-chunks of 128
G = D // 4                      # 1024 groups per token row

_compiled = None
_custom_ops = None


def _register_custom_dve():
    # Fused DVE prune op: out = |x| >= thr ? x : 0.
    global _custom_ops
    if _custom_ops is not None:
        return _custom_ops
    from concourse import dve_ops as Dv
    from concourse.dve_spec import (Spec, Src0, Src1, Zero, maxx, minn,
                                    select, lower)
    from concourse.dve_uop import DveOpSpec

    def mk(name, body, reference):
        spec = Spec(body=body, reference=reference)
        shas = {}
        for ver in ("v3", "v4"):
            try:
                u = lower(spec, ver=ver)
                shas[ver] = DveOpSpec(name=name, opcode=1, uops=u,
                                      rd1_en=True).sha(ver)
            except Exception:
                if ver == "v3":
                    raise
        return Dv.DveOp(name=name, spec=spec, subdim=False, uops_sha=shas)

    absa = maxx(Src0, Zero - Src0)
    absb = maxx(Src1, Zero - Src1)
    ops = (
        mk("PRUNE24_ANT", select(maxx(Src0, Zero - Src0) >= Src1, Src0, Zero),
           lambda in0, in1: np.where(np.abs(in0) >= in1, in0, 0.0)),
        mk("ABS_MAX2_ANT", maxx(absa, absb),
           lambda in0, in1: np.maximum(np.abs(in0), np.abs(in1))),
        mk("ABS_MIN2_ANT", minn(absa, absb),
           lambda in0, in1: np.minimum(np.abs(in0), np.abs(in1))),
    )
    for op in ops:
        if op.name not in Dv._SUB_OPCODE_FOR_NAME:
            Dv.OPS.append(op)
            Dv.CUSTOM_DVE_SPECS[op.name] = op.spec
            Dv._SUB_OPCODE_FOR_NAME[op.name] = (
                Dv._CUSTOM_DVE_ROW_BASE + len(Dv._SUB_OPCODE_FOR_NAME))
    _custom_ops = ops
    return ops


def _build():
    import concourse.tile as tile
    import concourse.mybir as mybir
    from concourse import bacc

    PRUNE24, ABS_MAX2, ABS_MIN2 = _register_custom_dve()
    f32 = mybir.dt.float32
    bf16 = mybir.dt.bfloat16
    i16 = mybir.dt.int16
    Alu = mybir.AluOpType

    nc = bacc.Bacc("TRN2", target_bir_lowering=False, debug=False,
                   num_devices=N_CORES)
    xs_ap = nc.dram_tensor("xs", [TOK, D], bf16, kind="ExternalInput").ap()
    # w arrives host-packed in the exact SBUF image layout
    # [partition, chunk*outf] so every DMA descriptor is a 16KB run
    wt_ap = nc.dram_tensor("wt", [P, NCH * OUTF], bf16,
                           kind="ExternalInput").ap()
    o_ap = nc.dram_tensor("o", [TOK, OUTF], f32, kind="ExternalOutput").ap()

    with tile.TileContext(nc) as tc:
        with tc.tile_pool(name="wpool", bufs=1) as wpool, \
             tc.tile_pool(name="xin", bufs=3) as xin, \
             tc.tile_pool(name="mwork", bufs=2) as mwork, \
             tc.tile_pool(name="xsp_p", bufs=4) as xsp_p, \
             tc.tile_pool(name="xtp", bufs=4) as xtp, \
             tc.tile_pool(name="outp", bufs=4) as outp, \
             tc.tile_pool(name="pso", bufs=8, space="PSUM") as pso:

            # weight.T (host-permuted, host-packed to the SBUF image)
            # resident in SBUF as [partition, chunk, outf].  1MB pieces with
            # one 16KB descriptor per partition ride both HWDGE rings and
            # all land before the first XBAR transpose is ready; gpsimd /
            # SWDGE stays empty so nothing gates the transposes.
            w_sb = wpool.tile([P, NCH, OUTF], bf16)
            WPC = 4 * OUTF                       # elements per 4-chunk piece

            def load_w_piece(eng, j):
                eng.dma_start(
                    out=w_sb[:, 4 * j:4 * (j + 1), :],
                    in_=wt_ap[:, j * WPC:(j + 1) * WPC])

            for j in range(6):
                load_w_piece(nc.sync, j)

            from concourse.tile import add_dep_helper
            ta_hist, tb_hist, mm_last_a, mm_last_b = [], [], [], []
            for i in range(NT):
                # x loads ride the ACT HWDGE ring; the SP ring keeps only
                # transpose-half-A + output stores so transposes start fast
                xh = xin.tile([P, D], bf16, tag="xh")
                nc.scalar.dma_start(out=xh, in_=xs_ap[i * P:(i + 1) * P, :])
                if i == 0:
                    # last two w pieces ride the ACT ring behind x tile 0
                    load_w_piece(nc.scalar, 6)
                    load_w_piece(nc.scalar, 7)
                # fused |.|-max / |.|-min custom DVE ops; block order is
                # [x0, x2, x1, x3] so halves pair (x0,x1) and (x2,x3)
                mx = mwork.tile([P, 2 * G], bf16, tag="mx")
                mn = mwork.tile([P, 2 * G], bf16, tag="mn")
                nc.vector._custom_dve(ABS_MAX2, out=mx, in0=xh[:, :2 * G],
                                      in1=xh[:, 2 * G:])
                nc.vector._custom_dve(ABS_MIN2, out=mn, in0=xh[:, :2 * G],
                                      in1=xh[:, 2 * G:])
                # thr = 2nd-largest |x| per group = max(min(maxes), max(mins))
                t1 = mwork.tile([P, G], bf16, tag="t1")
                t2 = mwork.tile([P, G], bf16, tag="t2")
                nc.vector.tensor_tensor(t1, mx[:, :G], mx[:, G:], Alu.min)
                nc.vector.tensor_tensor(t2, mn[:, :G], mn[:, G:], Alu.max)
                nc.vector.tensor_tensor(t1, t1, t2, Alu.max)
                # prune: xsp = |x| >= thr ? x : 0 (exact bf16 compare),
                # one op per 2-block half so each XBAR transpose reads a
                # whole tile (partial-slice reads/writes on the custom XBAR
                # DMA are not tracked correctly by the dep machinery)
                thr_b = t1.unsqueeze(1).broadcast_to([P, 2, G])
                xsp_a = xsp_p.tile([P, D // 2], bf16, tag="xsp_a")
                xsp_b = xsp_p.tile([P, D // 2], bf16, tag="xsp_b")
                pa = nc.vector._custom_dve(
                    PRUNE24,
                    out=xsp_a.rearrange("p (j g) -> p j g", j=2),
                    in0=xh[:, :D // 2].rearrange("p (j g) -> p j g", j=2),
                    in1=thr_b)
                pb = nc.vector._custom_dve(
                    PRUNE24,
                    out=xsp_b.rearrange("p (j g) -> p j g", j=2),
                    in0=xh[:, D // 2:].rearrange("p (j g) -> p j g", j=2),
                    in1=thr_b)
                # explicit WAR edges: the slot these prunes rewrite was read
                # by the transposes 3 iterations ago (pool bufs=3)
                if i >= 4:
                    add_dep_helper(pa.ins, ta_hist[i - 4].ins,
                                   reason="xsp slot WAR vs XBAR read")
                    add_dep_helper(pb.ins, tb_hist[i - 4].ins,
                                   reason="xsp slot WAR vs XBAR read")
                # transpose [tok, d] -> [d%128, d//128, tok] on the DMA XBAR,
                # in two halves so the first matmuls only wait on half A.
                # BOTH must stay on one queue: concurrent XBAR transposes on
                # different HWDGE rings corrupt each other (verified on hw).
                xspTa = xtp.tile([P, NCH // 2, P], bf16, tag="xspTa")
                xspTb = xtp.tile([P, NCH // 2, P], bf16, tag="xspTb")
                ta = nc.sync.dma_start(out=xspTa, in_=xsp_a, transpose=True)
                tb = nc.sync.dma_start(out=xspTb, in_=xsp_b, transpose=True)
                if i >= 4:
                    add_dep_helper(ta.ins, mm_last_a[i - 4].ins,
                                   reason="xspT slot WAR vs matmul read")
                    add_dep_helper(tb.ins, mm_last_b[i - 4].ins,
                                   reason="xspT slot WAR vs matmul read")
                ta_hist.append(ta)
                tb_hist.append(tb)
                # matmul: psum[tok, outf-half] += xspT[c].T @ wT[c]
                for n in range(2):
                    pout = pso.tile([P, OUTF // 2], f32)
                    for c in range(NCH):
                        xT = xspTa if c < NCH // 2 else xspTb
                        mm = nc.tensor.matmul(
                            pout,
                            xT[:, c % (NCH // 2), :],
                            w_sb[:, c, n * 512:(n + 1) * 512],
                            start=(c == 0), stop=(c == NCH - 1))
                        if n == 1 and c == NCH // 2 - 1:
                            mm_a = mm
                        elif n == 1 and c == NCH - 1:
                            mm_b = mm
                    osb = outp.tile([P, OUTF // 2], f32, tag="osb")
                    nc.scalar.copy(osb, pout)
                    # out stores ride the ACT ring: the sync ring carries
                    # only the XBAR transposes, so they never queue behind
                    # output traffic
                    nc.scalar.dma_start(
                        out=o_ap[i * P:(i + 1) * P, n * 512:(n + 1) * 512],
                        in_=osb)
                mm_last_a.append(mm_a)
                mm_last_b.append(mm_b)
    nc.compile()
    return nc


def _get_compiled():
    global _compiled
    if _compiled is None:
        _compiled = _build()
    return _compiled


def _fix_ties_bf16(xb32, x_orig):
    # The device keeps elements with |x| >= (2nd-largest |x| of the group),
    # evaluated on bf16 values.  When the 2nd and 3rd magnitudes round to
    # the same bf16 the device would keep 3+ elements, while the reference
    # (fp32 top_k, stable) keeps exactly 2.  Pre-zero the reference-dropped
    # elements of tied groups so the device selection matches exactly.
    g = np.abs(xb32.reshape(-1, 4))
    s = np.sort(g, axis=1)          # ascending: s[:,2]=2nd largest, s[:,1]=3rd
    tied = s[:, 2] == s[:, 1]
    if not tied.any():
        return xb32
    gv = xb32.reshape(-1, 4)
    go = x_orig.reshape(-1, 4)[tied]
    keep = np.argsort(-np.abs(go), axis=1, kind="stable")[:, :2]
    mask = np.zeros(go.shape, dtype=bool)
    np.put_along_axis(mask, keep, True, axis=1)
    sub = gv[tied]
    sub[~mask] = 0.0
    gv[tied] = sub
    return xb32


# group-position block order: pairs (x0,x1) and (x2,x3) land in opposite
# halves so the DVE tree pairs them with packed stride-1 slices
_BLOCK_ORDER = [0, 2, 1, 3]


def _prepare_inputs(x: np.ndarray, weight: np.ndarray) -> list:
    import ml_dtypes

    bo = _BLOCK_ORDER
    x_flat = np.ascontiguousarray(x.reshape(TOK_TOTAL, D), dtype=np.float32)
    xb32 = x_flat.astype(ml_dtypes.bfloat16).astype(np.float32)
    xb32 = _fix_ties_bf16(xb32, x_flat)
    # permute d so group position j sits in contiguous block bo.index(j)
    xp = np.ascontiguousarray(
        xb32.reshape(TOK_TOTAL, G, 4)[:, :, bo].transpose(0, 2, 1)
        .reshape(TOK_TOTAL, D)).astype(ml_dtypes.bfloat16)
    wtp = (weight.T.astype(np.float32).reshape(G, 4, OUTF)[:, bo, :]
           .transpose(1, 0, 2).reshape(D, OUTF))
    # pack to the SBUF image: [partition, chunk*outf], chunk-major per row
    wimg = np.ascontiguousarray(
        wtp.reshape(NCH, P, OUTF).transpose(1, 0, 2)
        .reshape(P, NCH * OUTF)).astype(ml_dtypes.bfloat16)
    return [{"xs": xp[c * TOK:(c + 1) * TOK], "wt": wimg}
            for c in range(N_CORES)]


def kernel(x: np.ndarray, weight: np.ndarray) -> np.ndarray:
    from concourse.bass_utils import run_bass_kernel_spmd

    nc = _get_compiled()
    in_maps = _prepare_inputs(x, weight)
    res = run_bass_kernel_spmd(nc, in_maps, core_ids=list(range(N_CORES)))
    out = np.concatenate([res.results[c]["o"] for c in range(N_CORES)], axis=0)
    return out.reshape(BS, SEQ, OUTF)


# revision 44
# speedup vs baseline: 1.1166x; 1.1166x over previous
# Trainium2 Bass kernel: 2:4 structured activation pruning + Linear.
#
#   out = magnitude_prune_2of4(x.reshape(-1, 4096)) @ weight.T
#
# Sharding: data-parallel over the flattened token dim (16384 tokens ->
# 2048/core across 8 cores); weight replicated. No collectives.
#
# Layout trick: the host casts x to bf16 and permutes the d axis so the
# four group positions land in four contiguous 1024-wide blocks, ordered
# [x0, x2, x1, x3].  Every DVE op in the 2:4 threshold tree then runs on
# packed stride-1 bf16 operands (2x_1p DVE mode), with no strided or
# broadcast access patterns.  The same permutation is applied to the
# weight rows on the host, so the GEMM is unchanged.
#
# Per-core pipeline, per 128-token tile:
#   DMA x (bf16) -> ACT Abs -> DVE stock max/min tree (packed, 2x) ->
#   per-group 2nd-max threshold (exact bf16) -> custom DVE prune
#   (|x| >= thr ? x : 0) -> DMA XBAR transpose SBUF->SBUF (bf16) ->
#   PE bf16 matmul accumulating over 32 d-chunks -> ACT PSUM->SBUF ->
#   DMA out.  The PE does nothing but the GEMM (no on-PE transposes).
import numpy as np

N_CORES = 8
BS, SEQ, D = 4, 4096, 4096
OUTF = 1024
TOK_TOTAL = BS * SEQ
TOK = TOK_TOTAL // N_CORES      # 2048 tokens per core
P = 128                         # SBUF partitions
NT = TOK // P                   # 16 token tiles per core
NCH = D // P                    # 32 d-chunks of 128
G = D // 4                      # 1024 groups per token row

_compiled = None
_custom_ops = None


def _register_custom_dve():
    # Fused DVE prune op: out = |x| >= thr ? x : 0.
    global _custom_ops
    if _custom_ops is not None:
        return _custom_ops
    from concourse import dve_ops as Dv
    from concourse.dve_spec import (Spec, Src0, Src1, Zero, maxx, minn,
                                    select, lower)
    from concourse.dve_uop import DveOpSpec

    def mk(name, body, reference):
        spec = Spec(body=body, reference=reference)
        shas = {}
        for ver in ("v3", "v4"):
            try:
                u = lower(spec, ver=ver)
                shas[ver] = DveOpSpec(name=name, opcode=1, uops=u,
                                      rd1_en=True).sha(ver)
            except Exception:
                if ver == "v3":
                    raise
        return Dv.DveOp(name=name, spec=spec, subdim=False, uops_sha=shas)

    absa = maxx(Src0, Zero - Src0)
    absb = maxx(Src1, Zero - Src1)
    ops = (
        mk("PRUNE24_ANT", select(maxx(Src0, Zero - Src0) >= Src1, Src0, Zero),
           lambda in0, in1: np.where(np.abs(in0) >= in1, in0, 0.0)),
        mk("ABS_MAX2_ANT", maxx(absa, absb),
           lambda in0, in1: np.maximum(np.abs(in0), np.abs(in1))),
        mk("ABS_MIN2_ANT", minn(absa, absb),
           lambda in0, in1: np.minimum(np.abs(in0), np.abs(in1))),
    )
    for op in ops:
        if op.name not in Dv._SUB_OPCODE_FOR_NAME:
            Dv.OPS.append(op)
            Dv.CUSTOM_DVE_SPECS[op.name] = op.spec
            Dv._SUB_OPCODE_FOR_NAME[op.name] = (
                Dv._CUSTOM_DVE_ROW_BASE + len(Dv._SUB_OPCODE_FOR_NAME))
    _custom_ops = ops
    return ops


def _build():
    import concourse.tile as tile
    import concourse.mybir as mybir
    from concourse import bacc

    PRUNE24, ABS_MAX2, ABS_MIN2 = _register_custom_dve()
    f32 = mybir.dt.float32
    bf16 = mybir.dt.bfloat16
    i16 = mybir.dt.int16
    Alu = mybir.AluOpType

    nc = bacc.Bacc("TRN2", target_bir_lowering=False, debug=False,
                   num_devices=N_CORES)
    xs_ap = nc.dram_tensor("xs", [TOK, D], bf16, kind="ExternalInput").ap()
    # w arrives host-packed in the exact SBUF image layout
    # [partition, chunk*outf] so every DMA descriptor is a 16KB run
    wt_ap = nc.dram_tensor("wt", [P, NCH * OUTF], bf16,
                           kind="ExternalInput").ap()
    o_ap = nc.dram_tensor("o", [TOK, OUTF], f32, kind="ExternalOutput").ap()

    with tile.TileContext(nc) as tc:
        with tc.tile_pool(name="wpool", bufs=1) as wpool, \
             tc.tile_pool(name="xin", bufs=4) as xin, \
             tc.tile_pool(name="mwork", bufs=2) as mwork, \
             tc.tile_pool(name="xsp_p", bufs=4) as xsp_p, \
             tc.tile_pool(name="xtp", bufs=4) as xtp, \
             tc.tile_pool(name="outp", bufs=2) as outp, \
             tc.tile_pool(name="pso", bufs=8, space="PSUM") as pso:

            # weight.T (host-permuted, host-packed to the SBUF image)
            # resident in SBUF as [partition, chunk, outf].  1MB pieces with
            # one 16KB descriptor per partition ride both HWDGE rings and
            # all land before the first XBAR transpose is ready; gpsimd /
            # SWDGE stays empty so nothing gates the transposes.
            w_sb = wpool.tile([P, NCH, OUTF], bf16)
            WPC = 4 * OUTF                       # elements per 4-chunk piece

            def load_w_piece(eng, j):
                eng.dma_start(
                    out=w_sb[:, 4 * j:4 * (j + 1), :],
                    in_=wt_ap[:, j * WPC:(j + 1) * WPC])

            for j in range(6):
                load_w_piece(nc.sync, j)

            from concourse.tile import add_dep_helper
            ta_hist, tb_hist, mm_last_a, mm_last_b = [], [], [], []
            for i in range(NT):
                # x loads ride the ACT HWDGE ring; the SP ring keeps only
                # transpose-half-A + output stores so transposes start fast
                xh = xin.tile([P, D], bf16, tag="xh")
                nc.scalar.dma_start(out=xh, in_=xs_ap[i * P:(i + 1) * P, :])
                if i == 0:
                    # last two w pieces ride the ACT ring behind x tile 0
                    load_w_piece(nc.scalar, 6)
                    load_w_piece(nc.scalar, 7)
                # fused |.|-max / |.|-min custom DVE ops; block order is
                # [x0, x2, x1, x3] so halves pair (x0,x1) and (x2,x3)
                mx = mwork.tile([P, 2 * G], bf16, tag="mx")
                mn = mwork.tile([P, 2 * G], bf16, tag="mn")
                nc.vector._custom_dve(ABS_MAX2, out=mx, in0=xh[:, :2 * G],
                                      in1=xh[:, 2 * G:])
                nc.vector._custom_dve(ABS_MIN2, out=mn, in0=xh[:, :2 * G],
                                      in1=xh[:, 2 * G:])
                # thr = 2nd-largest |x| per group = max(min(maxes), max(mins))
                t1 = mwork.tile([P, G], bf16, tag="t1")
                t2 = mwork.tile([P, G], bf16, tag="t2")
                nc.vector.tensor_tensor(t1, mx[:, :G], mx[:, G:], Alu.min)
                nc.vector.tensor_tensor(t2, mn[:, :G], mn[:, G:], Alu.max)
                nc.vector.tensor_tensor(t1, t1, t2, Alu.max)
                # prune: xsp = |x| >= thr ? x : 0 (exact bf16 compare),
                # one op per 2-block half so each XBAR transpose reads a
                # whole tile (partial-slice reads/writes on the custom XBAR
                # DMA are not tracked correctly by the dep machinery)
                thr_b = t1.unsqueeze(1).broadcast_to([P, 2, G])
                xsp_a = xsp_p.tile([P, D // 2], bf16, tag="xsp_a")
                xsp_b = xsp_p.tile([P, D // 2], bf16, tag="xsp_b")
                pa = nc.vector._custom_dve(
                    PRUNE24,
                    out=xsp_a.rearrange("p (j g) -> p j g", j=2),
                    in0=xh[:, :D // 2].rearrange("p (j g) -> p j g", j=2),
                    in1=thr_b)
                pb = nc.vector._custom_dve(
                    PRUNE24,
                    out=xsp_b.rearrange("p (j g) -> p j g", j=2),
                    in0=xh[:, D // 2:].rearrange("p (j g) -> p j g", j=2),
                    in1=thr_b)
                # explicit WAR edges: the slot these prunes rewrite was read
                # by the transposes 3 iterations ago (pool bufs=3)
                if i >= 4:
                    add_dep_helper(pa.ins, ta_hist[i - 4].ins,
                                   reason="xsp slot WAR vs XBAR read")
                    add_dep_helper(pb.ins, tb_hist[i - 4].ins,
                                   reason="xsp slot WAR vs XBAR read")
                # transpose [tok, d] -> [d%128, d//128, tok] on the DMA XBAR,
                # in two halves so the first matmuls only wait on half A.
                # BOTH must stay on one queue: concurrent XBAR transposes on
                # different HWDGE rings corrupt each other (verified on hw).
                xspTa = xtp.tile([P, NCH // 2, P], bf16, tag="xspTa")
                xspTb = xtp.tile([P, NCH // 2, P], bf16, tag="xspTb")
                ta = nc.sync.dma_start(out=xspTa, in_=xsp_a, transpose=True)
                tb = nc.sync.dma_start(out=xspTb, in_=xsp_b, transpose=True)
                if i >= 4:
                    add_dep_helper(ta.ins, mm_last_a[i - 4].ins,
                                   reason="xspT slot WAR vs matmul read")
                    add_dep_helper(tb.ins, mm_last_b[i - 4].ins,
                                   reason="xspT slot WAR vs matmul read")
                ta_hist.append(ta)
                tb_hist.append(tb)
                # matmul: psum[tok, outf-half] += xspT[c].T @ wT[c]
                for n in range(2):
                    pout = pso.tile([P, OUTF // 2], f32)
                    for c in range(NCH):
                        xT = xspTa if c < NCH // 2 else xspTb
                        mm = nc.tensor.matmul(
                            pout,
                            xT[:, c % (NCH // 2), :],
                            w_sb[:, c, n * 512:(n + 1) * 512],
                            start=(c == 0), stop=(c == NCH - 1))
                        if n == 1 and c == NCH // 2 - 1:
                            mm_a = mm
                        elif n == 1 and c == NCH - 1:
                            mm_b = mm
                    osb = outp.tile([P, OUTF // 2], f32, tag="osb")
                    nc.scalar.copy(osb, pout)
                    nc.sync.dma_start(
                        out=o_ap[i * P:(i + 1) * P, n * 512:(n + 1) * 512],
                        in_=osb)
                mm_last_a.append(mm_a)
                mm_last_b.append(mm_b)
    nc.compile()
    return nc


def _get_compiled():
    global _compiled
    if _compiled is None:
        _compiled = _build()
    return _compiled


def _fix_ties_bf16(xb32, x_orig):
    # The device keeps elements with |x| >= (2nd-largest |x| of the group),
    # evaluated on bf16 values.  When the 2nd and 3rd magnitudes round to
    # the same bf16 the device would keep 3+ elements, while the reference
    # (fp32 top_k, stable) keeps exactly 2.  Pre-zero the reference-dropped
    # elements of tied groups so the device selection matches exactly.
    g = np.abs(xb32.reshape(-1, 4))
    s = np.sort(g, axis=1)          # ascending: s[:,2]=2nd largest, s[:,1]=3rd
    tied = s[:, 2] == s[:, 1]
    if not tied.any():
        return xb32
    gv = xb32.reshape(-1, 4)
    go = x_orig.reshape(-1, 4)[tied]
    keep = np.argsort(-np.abs(go), axis=1, kind="stable")[:, :2]
    mask = np.zeros(go.shape, dtype=bool)
    np.put_along_axis(mask, keep, True, axis=1)
    sub = gv[tied]
    sub[~mask] = 0.0
    gv[tied] = sub
    return xb32


# group-position block order: pairs (x0,x1) and (x2,x3) land in opposite
# halves so the DVE tree pairs them with packed stride-1 slices
_BLOCK_ORDER = [0, 2, 1, 3]


def _prepare_inputs(x: np.ndarray, weight: np.ndarray) -> list:
    import ml_dtypes

    bo = _BLOCK_ORDER
    x_flat = np.ascontiguousarray(x.reshape(TOK_TOTAL, D), dtype=np.float32)
    xb32 = x_flat.astype(ml_dtypes.bfloat16).astype(np.float32)
    xb32 = _fix_ties_bf16(xb32, x_flat)
    # permute d so group position j sits in contiguous block bo.index(j)
    xp = np.ascontiguousarray(
        xb32.reshape(TOK_TOTAL, G, 4)[:, :, bo].transpose(0, 2, 1)
        .reshape(TOK_TOTAL, D)).astype(ml_dtypes.bfloat16)
    wtp = (weight.T.astype(np.float32).reshape(G, 4, OUTF)[:, bo, :]
           .transpose(1, 0, 2).reshape(D, OUTF))
    # pack to the SBUF image: [partition, chunk*outf], chunk-major per row
    wimg = np.ascontiguousarray(
        wtp.reshape(NCH, P, OUTF).transpose(1, 0, 2)
        .reshape(P, NCH * OUTF)).astype(ml_dtypes.bfloat16)
    return [{"xs": xp[c * TOK:(c + 1) * TOK], "wt": wimg}
            for c in range(N_CORES)]


def kernel(x: np.ndarray, weight: np.ndarray) -> np.ndarray:
    from concourse.bass_utils import run_bass_kernel_spmd

    nc = _get_compiled()
    in_maps = _prepare_inputs(x, weight)
    res = run_bass_kernel_spmd(nc, in_maps, core_ids=list(range(N_CORES)))
    out = np.concatenate([res.results[c]["o"] for c in range(N_CORES)], axis=0)
    return out.reshape(BS, SEQ, OUTF)
